# revision 22
# baseline (speedup 1.0000x reference)
"""DeepseekV3 decoder layer on 8 Trainium2 NeuronCores (Bass/Tile).

Sharding: sequence-parallel low-rank projections (one AllGather per latent
group), tensor-parallel heads for q_b/kv_b/attention (2 heads/core,
transposed-score layout), AllGather of head outputs, output-feature-sharded
o_proj + residual, AllGather of the raw post-attention hidden (post-LN stats
are recomputed locally on every core - no stats AllReduce), FF-sharded MLP
with per-chunk ReduceScatter.

Schedule: attention query chunks run in descending size order (3,2,1,0) so
the smallest chunk is last; q_b/rope for chunk j-1 and o_proj/AG3 for chunk
j+1 are interleaved between attention chunks; the MLP consumes AG3 chunks in
arrival order and the final down-proj/ReduceScatter is split 2x256 to shrink
the tail.

RMS scale-invariance: RMS() of a row of (x @ W) is independent of a
per-token scale on x, so the q/kv latent projections run directly on raw
bf16 x with no input RMS on the critical path; only k_pe (64 rows) needs
the 1/rms(x) factor. RMS/ln weights and the rope de-interleave are folded
into the weights host-side.
"""

import numpy as np

B, S, H = 1, 2048, 2048
NH, NOPE, ROPE, VHD = 16, 128, 64, 128
QHD = NOPE + ROPE
QLR, KVLR, FF = 1536, 512, 8192
SCALE = QHD ** -0.5
EPS = 1e-6
NC = 8
SS = S // NC            # 256: sequence / output-feature shard
FFS = FF // NC          # 1024: FF shard
P = 128

TRACE = False           # test.py sets kernel.TRACE = True for profiling

_CACHE = {}


def _tile_w(w):
    """[K, M] -> [K/128, ceil(M/128), 128, 128] contiguous blocks (zero-pad M)."""
    K, M = w.shape
    mc = -(-M // P)
    out = np.zeros((K // P, mc, P, P), np.float32)
    wp = np.zeros((K, mc * P), np.float32)
    wp[:, :M] = w
    for kt in range(K // P):
        for m in range(mc):
            out[kt, m] = wp[kt * P:(kt + 1) * P, m * P:(m + 1) * P]
    return out


def _build():
    if "nc" in _CACHE:
        return _CACHE["nc"]
    import concourse.mybir as mybir
    import concourse.tile as tile
    from concourse import bacc

    F32 = mybir.dt.float32
    F32R = mybir.dt.float32r
    BF16 = mybir.dt.bfloat16
    AF = mybir.ActivationFunctionType

    nc = bacc.Bacc("TRN2", target_bir_lowering=False, debug=False, num_devices=NC)

    def inp(name, shape, dt=F32):
        return nc.dram_tensor(name, list(shape), dt, kind="ExternalInput").ap()

    hT_s = inp("hT_s", [H, SS])
    hT_r = inp("hT_r", [SS, S])
    wq_a_t = inp("wq_a_t", [12, P, 16, P], BF16)
    wkv_a_t = inp("wkv_a_t", [5, P, 16, P], BF16)
    wqbn_t = inp("wqbn_t", [16, P, 12, P], BF16)
    wqbr_t = inp("wqbr_t", [8, P, 12, P], BF16)
    wkv_b_t = inp("wkv_b_t", [P, 4, 4, P], BF16)
    wo_t = inp("wo_t", [P, 16, 2, P], BF16)
    wg_t = inp("wg_t", [8, P, 16, P], BF16)
    wu_t = inp("wu_t", [8, P, 16, P], BF16)
    wd_t = inp("wd_t", [P, 8, 16, P], BF16)
    cs_sh = inp("cs_sh", [P, SS])             # rows 0:64 cosT, 64:128 signed sinT (own shard)
    cs2_sh = inp("cs2_sh", [2 * P, SS])       # rows 0:128 [cos;cos], 128:256 [sin;sin] (own shard)
    dmask = inp("dmask", [P, 4, 512])
    outT = nc.dram_tensor("outT", [SS, S], F32, kind="ExternalOutput").ap()

    RG = [list(range(NC))]

    from contextlib import ExitStack
    with tile.TileContext(nc) as tc, ExitStack() as _stack:
        cpool = _stack.enter_context(tc.tile_pool(name="const", bufs=1))
        dpool = _stack.enter_context(tc.tile_pool(name="dram", bufs=1, space="DRAM"))
        perm = _stack.enter_context(tc.tile_pool(name="perm", bufs=1))

        ag1a_in = dpool.tile([P, 5 * SS], BF16)
        ag1a_out = dpool.tile([NC * P, 5 * SS], BF16, addr_space="Shared")
        a2a_in = dpool.tile([NC * 384, SS], BF16)
        a2a_out = dpool.tile([NC * 384, SS], BF16, name="a2a_out")
        _ag2w = [1024, 512, 512]          # h1 (chunks 3+2), c1, c0
        ag2_in = [dpool.tile([2 * VHD, w], BF16, name=f"ag2_in{j}")
                  for j, w in enumerate(_ag2w)]
        ag2_out = [dpool.tile([NH * VHD, w], BF16, addr_space="Shared",
                              name=f"ag2_out{j}") for j, w in enumerate(_ag2w)]
        # per attn chunk j: (ag2 buffer index, col offset, oT col offset)
        AG2_MAP = {3: (0, 512, 1024), 2: (0, 0, 1024), 1: (1, 0, 512),
                   0: (2, 0, 0)}
        ag3_in = [dpool.tile([SS, 1024], BF16, name=f"ag3_in{j}") for j in range(2)]
        ag3_out = [dpool.tile([H, 1024], BF16, addr_space="Shared",
                              name=f"ag3_out{j}") for j in range(2)]
        # rs chunks: (outT column offset, width), in processing order
        RS_CH = [(1536, 512), (1024, 512), (512, 512), (0, 256), (256, 256)]
        rs_in = [dpool.tile([H, w], BF16, name=f"rs_in{j}")
                 for j, (c0, w) in enumerate(RS_CH)]
        rs_out = [dpool.tile([SS, w], BF16, name=f"rs_out{j}")
                  for j, (c0, w) in enumerate(RS_CH)]

        ones_f = cpool.tile([P, 1], F32)
        nc.vector.memset(ones_f[:], 1.0)
        ones_r = cpool.tile([P, 1], BF16)
        nc.vector.tensor_copy(ones_r[:], ones_f[:])
        eps_t = cpool.tile([P, 1], F32)
        nc.vector.memset(eps_t[:], EPS)
        ones_k1f = cpool.tile([1, P], F32)
        nc.vector.memset(ones_k1f[:], 1.0)
        ones_k1 = cpool.tile([1, P], F32R)
        nc.vector.tensor_copy(ones_k1[:], ones_k1f[:])

        h2 = perm.tile([P, 2, S], F32)        # post-attn hidden, own feature shard
        wos = perm.tile([P, 16, 2, P], BF16)  # o_proj weights (used in B and D)

        # shared psum pool for o_proj / stats / down accumulators (phases B+D)
        pbx = _stack.enter_context(tc.tile_pool(name="pbx", bufs=2, space="PSUM"))

        # ================= Stage A: seq-shard low-rank path =================
        with tc.tile_pool(name="sa", bufs=1) as sa, \
             tc.tile_pool(name="saw", bufs=5) as saw, \
             tc.tile_pool(name="pap", bufs=2, space="PSUM") as pa:
            with nc.named_scope("stageA"):
                xs = sa.tile([P, 16, SS], F32)
                nc.sync.dma_start(xs[:], hT_s.rearrange("(kt p) s -> p kt s", p=P))
                xb = sa.tile([P, 16, SS], BF16)
                for g in range(4):
                    nc.vector.tensor_copy(xb[:, 4 * g:4 * g + 4],
                                          xs[:, 4 * g:4 * g + 4])

                # kv latents on raw x (RMS scale-invariance)
                cvs = sa.tile([P, 5, SS], F32)
                for mc in range(5):
                    wt = saw.tile([P, 16, P], BF16, tag="aw")
                    nc.sync.dma_start(wt[:], wkv_a_t[mc])
                    ps = pa.tile([P, SS], F32, tag="amm")
                    for kt in range(16):
                        nc.tensor.matmul(ps[:], wt[:, kt], xb[:, kt],
                                         start=(kt == 0), stop=(kt == 15))
                    nc.vector.tensor_copy(cvs[:, mc], ps[:])

                # kv_a RMS (on raw latents; the 1/rms(x) factor cancels)
                sq3 = sa.tile([P, 4, SS], BF16)
                nc.vector.tensor_mul(sq3[:], cvs[:, :4], cvs[:, :4])
                msq3 = pa.tile([1, SS], F32, tag="acc", bufs=1)
                for mc in range(4):
                    nc.tensor.matmul(msq3[:], ones_r[:], sq3[:, mc],
                                     start=(mc == 0), stop=(mc == 3))
                r3s = sa.tile([1, SS], F32)
                nc.scalar.activation(r3s[:], msq3[:], AF.Sqrt, scale=1.0 / KVLR, bias=eps_t[:1])
                r3 = sa.tile([1, SS], F32R)
                with nc.allow_low_precision(reason="f32r rounding of rms scale"):
                    nc.vector.reciprocal(r3[:], r3s[:])
                r3bp = pa.tile([P, SS], F32, tag="rb", bufs=1)
                nc.tensor.matmul(r3bp[:], ones_k1[:], r3[:], start=True, stop=True)
                r3b = sa.tile([P, SS], F32)
                nc.vector.tensor_copy(r3b[:], r3bp[:])
                ckn = sa.tile([P, 4, SS], BF16)
                nc.vector.tensor_mul(ckn[:], cvs[:, :4],
                                     r3b[:, None, :].to_broadcast([P, 4, SS]))

                # rms(x) for the k_pe rows only
                sqx = sa.tile([P, 16, SS], BF16)
                nc.vector.tensor_mul(sqx[:], xb[:], xb[:])
                msq1 = pa.tile([1, SS], F32, tag="acc", bufs=1)
                for kt in range(16):
                    nc.tensor.matmul(msq1[:], ones_r[:], sqx[:, kt],
                                     start=(kt == 0), stop=(kt == 15))
                r1s = sa.tile([1, SS], F32)
                nc.scalar.activation(r1s[:], msq1[:], AF.Sqrt, scale=1.0 / H, bias=eps_t[:1])
                r1 = sa.tile([1, SS], F32R)
                with nc.allow_low_precision(reason="f32r rounding of rms scale"):
                    nc.vector.reciprocal(r1[:], r1s[:])
                r1bp = pa.tile([64, SS], F32, tag="rb", bufs=1)
                nc.tensor.matmul(r1bp[:], ones_k1[:, :64], r1[:], start=True, stop=True)
                r1b = sa.tile([64, SS], F32)
                nc.vector.tensor_copy(r1b[:], r1bp[:])

                # k_pe rope on cvs[:64, 4] (cs_sh rows 0:64 cos, 64:128 signed sin)
                cos_sh = sa.tile([64, SS], F32)
                nc.sync.dma_start(cos_sh[:], cs_sh[0:64, :])
                sin_sh = sa.tile([64, SS], F32)
                nc.sync.dma_start(sin_sh[:], cs_sh[64:128, :])
                ksw = sa.tile([64, SS], F32)
                nc.sync.dma_start(ksw[0:32, :], cvs[32:64, 4])
                nc.sync.dma_start(ksw[32:64, :], cvs[0:32, 4])
                kro = sa.tile([64, SS], F32)
                nc.vector.tensor_mul(kro[:], cvs[:64, 4], cos_sh[:])
                t1 = sa.tile([64, SS], F32)
                nc.vector.tensor_mul(t1[:], ksw[:], sin_sh[:])
                nc.vector.tensor_add(kro[:], kro[:], t1[:])
                kpe_n = sa.tile([64, SS], BF16)
                nc.vector.tensor_mul(kpe_n[:], kro[:], r1b[:])

                nc.sync.dma_start(
                    ag1a_in[:, 0:4 * SS].rearrange("p (kt s) -> p kt s", s=SS),
                    ckn[:])
                nc.sync.dma_start(ag1a_in[:64, 4 * SS:5 * SS], kpe_n[:])
                nc.gpsimd.collective_compute(
                    "AllGather", mybir.AluOpType.bypass, replica_groups=RG,
                    ins=[ag1a_in], outs=[ag1a_out])

                # q latents on raw x
                us = sa.tile([P, 12, SS], F32)
                for mc in range(12):
                    wt = saw.tile([P, 16, P], BF16, tag="aw")
                    nc.sync.dma_start(wt[:], wq_a_t[mc])
                    ps = pa.tile([P, SS], F32, tag="amm")
                    for kt in range(16):
                        nc.tensor.matmul(ps[:], wt[:, kt], xb[:, kt],
                                         start=(kt == 0), stop=(kt == 15))
                    nc.vector.tensor_copy(us[:, mc], ps[:])

                sq2 = sa.tile([P, 12, SS], BF16)
                nc.vector.tensor_mul(sq2[:], us[:], us[:])
                msq2 = pa.tile([1, SS], F32, tag="acc", bufs=1)
                for mc in range(12):
                    nc.tensor.matmul(msq2[:], ones_r[:], sq2[:, mc],
                                     start=(mc == 0), stop=(mc == 11))
                r2s = sa.tile([1, SS], F32)
                nc.scalar.activation(r2s[:], msq2[:], AF.Sqrt, scale=1.0 / QLR, bias=eps_t[:1])
                r2 = sa.tile([1, SS], F32R)
                with nc.allow_low_precision(reason="f32r rounding of rms scale"):
                    nc.vector.reciprocal(r2[:], r2s[:])
                r2bp = pa.tile([P, SS], F32, tag="rb", bufs=1)
                nc.tensor.matmul(r2bp[:], ones_k1[:], r2[:], start=True, stop=True)
                r2b = sa.tile([P, SS], F32)
                nc.vector.tensor_copy(r2b[:], r2bp[:])
                un = sa.tile([P, 12, SS], BF16)
                nc.vector.tensor_mul(un[:], us[:],
                                     r2b[:, None, :].to_broadcast([P, 12, SS]))

                # q_b for ALL heads on own tokens, rope, then AllToAll
                qnb = sa.tile([P, 16, SS], BF16)
                for m in range(16):
                    wt = saw.tile([P, 12, P], BF16, tag="qw")
                    nc.sync.dma_start(wt[:], wqbn_t[m])
                    ps = pa.tile([P, SS], F32, tag="amm")
                    for kt in range(12):
                        nc.tensor.matmul(ps[:], wt[:, kt], un[:, kt],
                                         start=(kt == 0), stop=(kt == 11))
                    nc.vector.tensor_copy(qnb[:, m], ps[:])

                cs2c = sa.tile([P, SS], F32)
                nc.sync.dma_start(cs2c[:], cs2_sh[0:P, :])
                cs2s = sa.tile([P, SS], F32)
                nc.sync.dma_start(cs2s[:], cs2_sh[P:2 * P, :])
                qrb = sa.tile([P, 8, SS], BF16)
                for mr in range(8):
                    wt = saw.tile([P, 12, P], BF16, tag="qw")
                    nc.sync.dma_start(wt[:], wqbr_t[mr])
                    ps = pa.tile([P, SS], F32, tag="amm")
                    for kt in range(12):
                        nc.tensor.matmul(ps[:], wt[:, kt], un[:, kt],
                                         start=(kt == 0), stop=(kt == 11))
                    qr = sa.tile([P, SS], F32, tag="qr", bufs=2)
                    nc.vector.tensor_copy(qr[:], ps[:])
                    qsw = sa.tile([P, SS], F32, tag="qsw", bufs=2)
                    for qq in range(2):
                        b0 = qq * 64
                        nc.sync.dma_start(qsw[b0:b0 + 32, :],
                                          qr[b0 + 32:b0 + 64, :])
                        nc.sync.dma_start(qsw[b0 + 32:b0 + 64, :],
                                          qr[b0:b0 + 32, :])
                    qc1 = sa.tile([P, SS], F32, tag="qc1", bufs=2)
                    nc.vector.tensor_mul(qc1[:], qr[:], cs2c[:])
                    qs1 = sa.tile([P, SS], F32, tag="qs1", bufs=2)
                    nc.vector.tensor_mul(qs1[:], qsw[:], cs2s[:])
                    nc.vector.tensor_add(qrb[:, mr], qc1[:], qs1[:])

                for r in range(NC):
                    nc.scalar.dma_start(
                        a2a_in[384 * r:384 * r + 256, :].rearrange(
                            "(m p) s -> p m s", p=P),
                        qnb[:, 2 * r:2 * r + 2])
                    nc.scalar.dma_start(
                        a2a_in[384 * r + 256:384 * (r + 1), :], qrb[:, r])
                nc.gpsimd.collective_compute(
                    "AllToAll", mybir.AluOpType.bypass, replica_groups=RG,
                    ins=[a2a_in], outs=[a2a_out])

        _hooks = {}

        # ===== Stage B: kv_b all blocks, per-chunk q_b/rope + attention =====
        with tc.tile_pool(name="sb2", bufs=1) as sb2, \
             tc.tile_pool(name="sbr", bufs=1) as sbr, \
             tc.tile_pool(name="sbe", bufs=1) as sbe, \
             tc.tile_pool(name="scr", bufs=2) as scr:
            kT = sb2.tile([P, 2, S], BF16)
            kpeT = sb2.tile([64, S], BF16)
            v_tok = sb2.tile([P, 2, 16, P], BF16)
            qT = sb2.tile([P, 2, S], BF16)
            qpe2 = sb2.tile([64, 2, S], BF16)
            oT = sb2.tile([P, 2, S], BF16)
            wkb = sb2.tile([P, 4, 4, P], BF16)
            mask_t = sb2.tile([P, 4, 512], F32)
            nc.sync.dma_start(wkb[:], wkv_b_t[:])
            nc.sync.dma_start(mask_t[:], dmask[:, :, :])
            nc.sync.dma_start(wos[:], wo_t[:])

            def q_fill(pr):
                """Assemble qT/qpe2 for pair-block pr from the q AllToAll."""
                for b in range(2):
                    blk = 2 * pr + b
                    bsl = slice(blk * SS, (blk + 1) * SS)
                    nc.scalar.dma_start(
                        qT[:, :, bsl],
                        a2a_out[384 * blk:384 * blk + 256, :].rearrange(
                            "(m p) s -> p m s", p=P))
                    nc.scalar.dma_start(
                        qpe2[:, :, bsl],
                        a2a_out[384 * blk + 256:384 * (blk + 1), :].rearrange(
                            "(h p) s -> p h s", p=64))

            def oproj_fetch(j, pool, tags, nbufs=2):
                bi, boff, _ = AG2_MAP[j]
                nsl = slice(j * 512, (j + 1) * 512)
                rhs = pool.tile([P, 16, 512], BF16, tag=tags[0], name="ojr",
                                bufs=nbufs)
                nc.sync.dma_start(
                    rhs[:],
                    ag2_out[bi].rearrange("(kt p) s -> p kt s", p=P)[
                        :, :, boff:boff + 512])
                resid = pool.tile([P, 2, 512], F32, tag=tags[1], name="ojs",
                                  bufs=nbufs)
                nc.sync.dma_start(
                    resid[:],
                    hT_r.rearrange("(mc p) s -> p mc s", p=P)[:, :, nsl])
                return rhs, resid

            def oproj_compute(j, rhs, resid):
                nsl = slice(j * 512, (j + 1) * 512)
                for mc in range(2):
                    ps = pbx.tile([P, 512], F32, tag="big")
                    for kt in range(16):
                        nc.tensor.matmul(ps[:], wos[:, kt, mc], rhs[:, kt],
                                         start=(kt == 0), stop=(kt == 15))
                    nc.vector.tensor_add(h2[:, mc, nsl], ps[:], resid[:, mc])

            def h2b_flush(j, pool, tag):
                hf = j // 2
                off = j * 512 - hf * 1024
                nsl = slice(j * 512, (j + 1) * 512)
                h2b = pool.tile([P, 2, 512], BF16, tag=tag, name="h2b")
                nc.vector.tensor_copy(h2b[:], h2[:, :, nsl])
                nc.sync.dma_start(
                    ag3_in[hf].rearrange("(mc p) s -> p mc s", p=P)[
                        :, :, off:off + 512], h2b[:])

            def oproj(j):
                rhs, resid = oproj_fetch(j, scr, ("rhs2", "resid"))
                oproj_compute(j, rhs, resid)

            def ag3_go(hf):
                nc.gpsimd.collective_compute(
                    "AllGather", mybir.AluOpType.bypass, replica_groups=RG,
                    ins=[ag3_in[hf]], outs=[ag3_out[hf]])

            with tc.tile_pool(name="pbq", bufs=2, space="PSUM") as pbq, \
                 tc.tile_pool(name="pbo", bufs=2, space="PSUM") as pbo:
                with nc.named_scope("stageB_kv"):
                    for pr in range(4):
                        psl = slice(pr * 512, (pr + 1) * 512)
                        rhs_c = sbr.tile([P, 4, 2, SS], BF16, tag="rhs1c", bufs=2)
                        for b in range(2):
                            blk = 2 * pr + b
                            nc.sync.dma_start(
                                rhs_c[:, :, b, :],
                                ag1a_out[blk * P:(blk + 1) * P, 0:4 * SS].rearrange(
                                    "p (kt s) -> p kt s", s=SS))
                            nc.sync.dma_start(
                                kpeT[:, blk * SS:(blk + 1) * SS],
                                ag1a_out[blk * P:blk * P + 64, 4 * SS:5 * SS])
                        # k_nope (dim-major)
                        for mc in range(2):
                            ps = pbq.tile([P, 512], F32, tag="sc")
                            for kt in range(4):
                                nc.tensor.matmul(
                                    ps[:], wkb[:, kt, mc],
                                    rhs_c[:, kt].rearrange("p b s -> p (b s)"),
                                    start=(kt == 0), stop=(kt == 3))
                            nc.vector.tensor_copy(kT[:, mc, psl], ps[:])
                        # V token-major: stationary = latent tile, moving = v-cols
                        for b in range(2):
                            for st2 in range(2):
                                stile = pr * 4 + b * 2 + st2
                                pv = pbo.tile([P, 2, P], F32, tag="o")
                                for kt in range(4):
                                    nc.tensor.matmul(
                                        pv[:].rearrange("p h v -> p (h v)"),
                                        rhs_c[:, kt, b, st2 * P:(st2 + 1) * P],
                                        wkb[:, kt, 2:4, :].rearrange("p h v -> p (h v)"),
                                        start=(kt == 0), stop=(kt == 3))
                                nc.vector.tensor_copy(v_tok[:, :, stile, :], pv[:])

                def attn_chunk(qc):
                    qsl = slice(qc * 512, (qc + 1) * 512)
                    nkt = 4 * qc + 4
                    for h in range(2):
                        o_ps = pbo.tile([P, 512], F32, tag="o")
                        d_ps = pbx.tile([1, 512], F32, tag="acc")
                        for kt in range(nkt):
                            ksl = slice(kt * P, (kt + 1) * P)
                            sc_ps = pbq.tile([P, 512], F32, tag="sc")
                            nc.tensor.matmul(sc_ps[:], kT[:, h, ksl],
                                             qT[:, h, qsl], start=True, stop=False)
                            nc.tensor.matmul(sc_ps[:], kpeT[:, ksl],
                                             qpe2[:, h, qsl], start=False, stop=True)
                            j = kt - 4 * qc
                            if j >= 0:
                                nc.vector.tensor_add(sc_ps[:], sc_ps[:],
                                                     mask_t[:, j])
                            es = sbe.tile([P, 512], BF16, tag="es", bufs=4)
                            nc.scalar.activation(es[:], sc_ps[:], AF.Exp)
                            nc.tensor.matmul(o_ps[:], v_tok[:, h, kt], es[:],
                                             start=(kt == 0), stop=(kt == nkt - 1))
                            nc.tensor.matmul(d_ps[:], ones_r[:], es[:],
                                             start=(kt == 0), stop=(kt == nkt - 1))
                        rec = sbe.tile([1, 512], F32R, tag="rec", bufs=2)
                        with nc.allow_low_precision(
                                reason="f32r rounding of softmax denom"):
                            nc.vector.reciprocal(rec[:], d_ps[:])
                        rb_ps = pbx.tile([P, 512], F32, tag="big")
                        nc.tensor.matmul(rb_ps[:], ones_k1[:], rec[:],
                                         start=True, stop=True)
                        recb = sbe.tile([P, 512], F32, tag="recb", bufs=2)
                        nc.vector.tensor_copy(recb[:], rb_ps[:])
                        nc.vector.tensor_mul(oT[:, h, qsl], o_ps[:], recb[:])

                def ag2_go(bi, o_off, w):
                    nc.sync.dma_start(
                        ag2_in[bi].rearrange("(mc p) s -> p mc s", p=P),
                        oT[:, :, o_off:o_off + w])
                    nc.gpsimd.collective_compute(
                        "AllGather", mybir.AluOpType.bypass, replica_groups=RG,
                        ins=[ag2_in[bi]], outs=[ag2_out[bi]])

                with nc.named_scope("stageB_attn"):
                    q_fill(3)
                    q_fill(2)
                    attn_chunk(3)
                    q_fill(1)
                    attn_chunk(2)
                    ag2_go(0, 1024, 1024)
                    q_fill(0)
                    attn_chunk(1)
                    oproj(3)
                    oproj(2)
                    h2b_flush(3, scr, "h2b")
                    h2b_flush(2, scr, "h2b")
                    ag2_go(1, 512, 512)
                    attn_chunk(0)
                    ag3_go(1)
                    f1 = oproj_fetch(1, scr, ("rhs2", "resid"))
                    ag2_go(2, 0, 512)
                    oproj_compute(1, *f1)

            _hooks["oproj_fetch"] = oproj_fetch
            _hooks["oproj_compute"] = oproj_compute
            _hooks["h2b_flush"] = h2b_flush
            _hooks["ag3_go"] = ag3_go

        # ================= Stage D: post-LN + MLP, chunk pipelined =============
        with tc.tile_pool(name="wmlp", bufs=1) as wmlp, \
             tc.tile_pool(name="wstr", bufs=1) as wstr, \
             tc.tile_pool(name="smy", bufs=1) as smy, \
             tc.tile_pool(name="sdd", bufs=2) as sdd, \
             tc.tile_pool(name="pgu", bufs=1, space="PSUM") as pgu:
            with nc.named_scope("stageD"):
                wds = wmlp.tile([P, 8, 16, P], BF16)

                def hy_fetch(j):
                    hf = j // 2
                    off = j * 512 - hf * 1024
                    hy = smy.tile([P, 16, 512], BF16, tag="hy", bufs=2)
                    nc.sync.dma_start(
                        hy[:],
                        ag3_out[hf].rearrange("(kt p) s -> p kt s", p=P)[
                            :, :, off:off + 512])
                    return hy

                def stats(hy):
                    """Local post-LN stats for a chunk -> r4b broadcast tile.
                    The 1/rms scale is applied at gate/up/down psum readout
                    (it commutes through the linear layers)."""
                    m4 = pbx.tile([1, 512], F32, tag="acc")
                    for half in range(2):
                        sqh = smy.tile([P, 8, 512], BF16, tag="sqh", bufs=1)
                        nc.vector.tensor_mul(sqh[:], hy[:, half * 8:half * 8 + 8],
                                             hy[:, half * 8:half * 8 + 8])
                        for kt in range(8):
                            nc.tensor.matmul(m4[:], ones_r[:], sqh[:, kt],
                                             start=(half == 0 and kt == 0),
                                             stop=(half == 1 and kt == 7))
                    r4s = smy.tile([1, 512], F32, tag="r4s", bufs=2)
                    nc.scalar.activation(r4s[:], m4[:], AF.Sqrt,
                                         scale=1.0 / H, bias=eps_t[:1])
                    r4 = smy.tile([1, 512], F32R, tag="r4", bufs=2)
                    with nc.allow_low_precision(reason="f32r rounding of rms scale"):
                        nc.vector.reciprocal(r4[:], r4s[:])
                    r4bp = pbx.tile([P, 512], F32, tag="big")
                    nc.tensor.matmul(r4bp[:], ones_k1[:], r4[:],
                                     start=True, stop=True)
                    r4b = smy.tile([P, 512], F32, tag="r4b", bufs=2)
                    nc.vector.tensor_copy(r4b[:], r4bp[:])
                    return r4b

                def gateup(y, r4b, y_off, cw):
                    """gate/up on raw y[:, :, y_off:y_off+cw]; r4 scale applied
                    to the silu input (the up-branch scale rides to down)."""
                    act = smy.tile([P, 8, 512], BF16, tag="act", bufs=1,
                                   name="act")[:, :, :cw]
                    for m in range(8):
                        wgm = wstr.tile([P, 16, P], BF16, tag="wg", bufs=3)
                        nc.sync.dma_start(wgm[:], wg_t[m])
                        wum = wstr.tile([P, 16, P], BF16, tag="wu", bufs=3)
                        nc.sync.dma_start(wum[:], wu_t[m])
                        gp = pgu.tile([P, 512], F32, tag=f"g{m % 2}",
                                      name="gp")[:, :cw]
                        up = pgu.tile([P, 512], F32, tag=f"u{m % 2}",
                                      name="up")[:, :cw]
                        for kt in range(16):
                            nc.tensor.matmul(gp[:], wgm[:, kt],
                                             y[:, kt, y_off:y_off + cw],
                                             start=(kt == 0), stop=(kt == 15))
                            nc.tensor.matmul(up[:], wum[:, kt],
                                             y[:, kt, y_off:y_off + cw],
                                             start=(kt == 0), stop=(kt == 15))
                        t1 = smy.tile([P, 512], F32, tag="t1", bufs=2,
                                      name="t1")[:, :cw]
                        nc.vector.tensor_mul(t1[:], gp[:],
                                             r4b[:, y_off:y_off + cw])
                        gsil = sdd.tile([P, 512], BF16, tag="gsil",
                                        name="gsil")[:, :cw]
                        nc.scalar.activation(gsil[:], t1[:], AF.Silu)
                        nc.vector.tensor_mul(act[:, m], gsil[:], up[:])
                    return act

                def down(act, r4b, act_off, ri, r4_off=None):
                    """down-proj of act[:, :, act_off:act_off+w] -> RS chunk ri;
                    applies the deferred up-branch r4 scale at psum readout."""
                    c0, cw = RS_CH[ri]
                    if r4_off is None:
                        r4_off = act_off
                    nsl = slice(c0, c0 + cw)
                    for q in range(4):
                        dn = sdd.tile([P, 4, 512], BF16, tag="dn",
                                      name="dn")[:, :, :cw]
                        for s in range(4):
                            mc = 4 * q + s
                            ps = pbx.tile([P, 512], F32, tag="big",
                                          name="dps")[:, :cw]
                            for kt in range(8):
                                nc.tensor.matmul(
                                    ps[:], wds[:, kt, mc],
                                    act[:, kt, act_off:act_off + cw],
                                    start=(kt == 0), stop=(kt == 7))
                            nc.vector.tensor_mul(
                                dn[:, s], ps[:], r4b[:, r4_off:r4_off + cw])
                        nc.sync.dma_start(
                            rs_in[ri][4 * q * P:4 * (q + 1) * P, :].rearrange(
                                "(q p) s -> p q s", p=P), dn[:])
                    nc.gpsimd.collective_compute(
                        "ReduceScatter", mybir.AluOpType.add, replica_groups=RG,
                        ins=[rs_in[ri]], outs=[rs_out[ri]])
                    fin = sdd.tile([P, 2, 512], BF16, tag="fin",
                                   name="fin")[:, :, :cw]
                    nc.sync.dma_start(
                        fin[:], rs_out[ri].rearrange("(mc p) s -> p mc s", p=P))
                    fino = sdd.tile([P, 2, 512], F32, tag="fino", name="fino",
                                    bufs=1)[:, :, :cw]
                    nc.vector.tensor_add(fino[:], fin[:], h2[:, :, nsl])
                    nc.sync.dma_start(
                        outT.rearrange("(mc p) s -> p mc s", p=P)[:, :, nsl],
                        fino[:])

                hy3 = hy_fetch(3)
                r43 = stats(hy3)
                act3 = gateup(hy3, r43, 0, 512)
                nc.sync.dma_start(wds[:], wd_t[:])
                f0 = _hooks["oproj_fetch"](0, sdd, ("rhs2d", "residd"), nbufs=1)
                _hooks["oproj_compute"](0, *f0)
                hy2 = hy_fetch(2)
                r42 = stats(hy2)
                down(act3, r43, 0, 0)
                _hooks["h2b_flush"](1, sdd, "h2bd")
                _hooks["h2b_flush"](0, sdd, "h2bd")
                _hooks["ag3_go"](0)
                act2 = gateup(hy2, r42, 0, 512)
                hy1 = hy_fetch(1)
                r41 = stats(hy1)
                down(act2, r42, 0, 1)
                act1 = gateup(hy1, r41, 0, 512)
                hy0 = hy_fetch(0)
                r40 = stats(hy0)
                down(act1, r41, 0, 2)
                act0a = gateup(hy0, r40, 0, 256)
                down(act0a, r40, 0, 3)
                act0b = gateup(hy0, r40, 256, 256)
                down(act0b, r40, 0, 4, r4_off=256)

    nc.compile()
    _CACHE["nc"] = nc
    return nc


def _host_prep(inputs):
    import ml_dtypes
    bf16 = ml_dtypes.bfloat16
    inp = {k: np.asarray(v) for k, v in inputs.items()}
    hidden = inp["hidden_states"].reshape(S, H).astype(np.float32)
    pos = inp["position_ids"].reshape(S).astype(np.int64)
    cosT = inp["cos"][pos].T.astype(np.float32)
    sinT = inp["sin"][pos].T.astype(np.float32)
    wq_a = (inp["wq_a"] * inp["in_ln"][:, None]).astype(np.float32)
    wkv_a = (inp["wkv_a"] * inp["in_ln"][:, None]).astype(np.float32)
    wq_b = (inp["wq_b"] * inp["q_a_ln"][:, None]).astype(np.float32)
    wkv_b = (inp["wkv_b"] * inp["kv_a_ln"][:, None]).astype(np.float32)
    wg = (inp["w_gate"] * inp["post_ln"][:, None]).astype(np.float32)
    wu = (inp["w_up"] * inp["post_ln"][:, None]).astype(np.float32)
    wd = inp["w_down"].astype(np.float32)
    wo = inp["wo"].astype(np.float32)

    de = np.empty(ROPE, np.int64)
    de[:32] = np.arange(32) * 2
    de[32:] = np.arange(32) * 2 + 1
    wkv_a = np.concatenate([wkv_a[:, :KVLR], wkv_a[:, KVLR:][:, de]], axis=1)
    wq_b = wq_b.reshape(QLR, NH, QHD)
    wkv_b = wkv_b.reshape(KVLR, NH, NOPE + VHD)

    hT = hidden.T.copy()
    sin_sg = np.concatenate([-sinT[:32], sinT[32:]], axis=0)    # signed for swap trick
    qbn = (wq_b[:, :, :NOPE] * SCALE).reshape(QLR, NH * NOPE)
    qbr = (wq_b[:, :, NOPE:][:, :, de] * SCALE).reshape(QLR, NH * ROPE)
    wqbn_t = np.ascontiguousarray(
        _tile_w(qbn.astype(np.float32)).transpose(1, 2, 0, 3)).astype(bf16)
    wqbr_t = np.ascontiguousarray(
        _tile_w(qbr.astype(np.float32)).transpose(1, 2, 0, 3)).astype(bf16)
    ki = np.arange(P)[:, None]
    qi = np.arange(512)[None, :]
    dmask = np.stack([np.where(qi >= j * P + ki, 0.0, -1e30).astype(np.float32)
                      for j in range(4)], axis=1)               # (128, 4, 512)

    wq_a_t = _tile_w(wq_a)
    wkv_a_t = _tile_w(wkv_a)

    in_maps = []
    for c in range(NC):
        h0, h1 = 2 * c, 2 * c + 1
        kb = np.concatenate([
            wkv_b[:, h0, :NOPE], wkv_b[:, h1, :NOPE],
            wkv_b[:, h0, NOPE:], wkv_b[:, h1, NOPE:]], axis=1)
        ssl = slice(c * SS, (c + 1) * SS)
        cs_sh = np.concatenate([cosT[:, ssl], sin_sg[:, ssl]], axis=0)
        in_maps.append({
            "hT_s": np.ascontiguousarray(hT[:, ssl]),
            "hT_r": np.ascontiguousarray(hT[ssl, :]),
            "wq_a_t": np.ascontiguousarray(
                wq_a_t.transpose(1, 2, 0, 3)).astype(bf16),
            "wkv_a_t": np.ascontiguousarray(
                wkv_a_t.transpose(1, 2, 0, 3)).astype(bf16),
            "wqbn_t": wqbn_t,
            "wqbr_t": wqbr_t,
            "wkv_b_t": np.ascontiguousarray(_tile_w(
                kb.astype(np.float32)).transpose(2, 0, 1, 3)).astype(bf16),
            "wo_t": np.ascontiguousarray(_tile_w(np.ascontiguousarray(
                wo[:, ssl])).transpose(2, 0, 1, 3)).astype(bf16),
            "wg_t": np.ascontiguousarray(_tile_w(
                wg[:, c * FFS:(c + 1) * FFS]).transpose(1, 2, 0, 3)).astype(bf16),
            "wu_t": np.ascontiguousarray(_tile_w(
                wu[:, c * FFS:(c + 1) * FFS]).transpose(1, 2, 0, 3)).astype(bf16),
            "wd_t": np.ascontiguousarray(_tile_w(
                wd[c * FFS:(c + 1) * FFS, :]).transpose(2, 0, 1, 3)).astype(bf16),
            "cs_sh": np.ascontiguousarray(cs_sh),
            "cs2_sh": np.ascontiguousarray(np.concatenate(
                [cosT[:, ssl], cosT[:, ssl], sin_sg[:, ssl], sin_sg[:, ssl]],
                axis=0)),
            "dmask": dmask,
        })
    return in_maps


_LAST_RESULT = {}


def kernel(**inputs) -> np.ndarray:
    from concourse.bass_utils import run_bass_kernel_spmd
    nc = _build()
    in_maps = _host_prep(inputs)
    kwargs = {}
    if TRACE:
        import sys, types
        if "antenv.axon_hooks" not in sys.modules:
            try:
                from trn_agent_boot.trn_boot import _ntff_profile_via_ctypes
                mod = types.ModuleType("antenv.axon_hooks")
                _hook = _ntff_profile_via_ctypes('/opt/axon/libaxon_pjrt.so')
                mod.get_axon_ntff_profile_hook = lambda: _hook
                mod.set_axon_ntff_profile_hook = lambda h: None
                sys.modules["antenv.axon_hooks"] = mod
                import antenv
                antenv.axon_hooks = mod
            except Exception:
                pass
        kwargs["trace"] = True
    res = run_bass_kernel_spmd(nc, in_maps, list(range(NC)), **kwargs)
    _LAST_RESULT["res"] = res
    outT = np.concatenate([res.results[c]["outT"] for c in range(NC)], axis=0)
    return np.ascontiguousarray(outT.T)[None].astype(np.float32)
